# revision 5
# baseline (speedup 1.0000x reference)
"""MultiHeadAttention Trainium2 kernel (8 NeuronCores, SPMD).

Problem: B=2, L=2048, DK=DV=512, H=8, dh=64.
  Q = q @ WQ[h]; K = k @ WK[h]; V = v @ WV[h]       (per head)
  y = Q K^T / sqrt(L); z = softmax(y, axis=QUERY); out = z @ V
  concat heads on feature dim.

Sharding: 16 (b,h) pairs over 8 cores -> 2 heads (same batch) per core.

Device structure (per core, heads h0/h1):
  QT/KT = [e-pack(128), L] bf16; scores S[k-tile(128), q] in PSUM.
  exp is the bottleneck: split across ACT (exact Exp + fused accum row
  sums for D) and DVE (Schraudolph bf16-domain approx: one tensor_scalar
  computing round(score*A + B) -> int16, whose bit pattern read as bf16
  is exp(score*SCALE) to ~1.8% rel; D via tensor_reduce on the same
  tile).  1/D[k] is folded into V rows.
  AV uses E-tiles as the stationary operand: out[q(128), ev] accumulates
  over k-tiles in a single 4-bank PSUM region, 64 moving rows per
  matmul, so AV streams half the rows of the out^T[ev, q] layout.
Core output: [L, 128] f32 = two heads' outputs concatenated on the
feature dim; host slices it straight into the full output.
"""

import math

import numpy as np

B = 2
L = 2048
DK = 512
H = 8
DH = 64
P = 128
NKT = L // P  # 16 k-tiles
NDC = DK // P  # 4 d-chunks
N_CORES = 8

SCALE = 1.0 / math.sqrt(float(L))
# Schraudolph in bf16-bits domain: round(raw_score*EXP_A + EXP_B) as int16,
# bitcast bf16 ~= exp(raw_score*SCALE).  C calibrated against np.exp.
EXP_A = 128.0 * math.log2(math.e) * SCALE
EXP_B = 16256.0 - 12.0

# (kt, h, half) half-tiles whose exp runs on DVE instead of ACT.
DVE_TILES = {(kt, 1, 1) for kt in range(NKT)} | {(5, 0, 1), (10, 0, 1), (15, 0, 1)}

_CACHE = {}


def _build_program():
    import concourse.bass as bass
    import concourse.tile as tile
    from concourse import bacc, mybir
    from concourse.bass import ts

    f32 = mybir.dt.float32
    bf16 = mybir.dt.bfloat16
    i16 = mybir.dt.int16
    AF = mybir.ActivationFunctionType
    ALU = mybir.AluOpType

    nc = bacc.Bacc("TRN2", target_bir_lowering=False, debug=False)

    qt_d = nc.dram_tensor("qt", [DK, L], bf16, kind="ExternalInput")
    kt_d = nc.dram_tensor("kt", [DK, L], bf16, kind="ExternalInput")
    vt_d = nc.dram_tensor("vt", [DK, L], bf16, kind="ExternalInput")
    wq_d = nc.dram_tensor("wq", [DK, P], bf16, kind="ExternalInput")
    wk_d = nc.dram_tensor("wk", [DK, P], bf16, kind="ExternalInput")
    wv_d = nc.dram_tensor("wv", [DK, P], bf16, kind="ExternalInput")
    out_d = nc.dram_tensor("out", [L, P], f32, kind="ExternalOutput")

    with tile.TileContext(nc) as tc:
        with (
            tc.tile_pool(name="consts", bufs=1) as consts,
            tc.tile_pool(name="xin", bufs=1) as xin,
            tc.tile_pool(name="proj", bufs=1) as proj,
            tc.tile_pool(name="epool", bufs=4) as epool,
            tc.tile_pool(name="vspool", bufs=3) as vspool,
            tc.tile_pool(name="stats", bufs=1) as stats,
            tc.tile_pool(name="outp", bufs=2) as outp,
            tc.tile_pool(name="spsum", bufs=2, space="PSUM") as spsum,
            tc.tile_pool(name="avpsum", bufs=1, space="PSUM") as avpsum,
        ):
            wq_s = consts.tile([P, NDC, P], bf16)
            wk_s = consts.tile([P, NDC, P], bf16)
            wv_s = consts.tile([P, NDC, P], bf16)
            qt_s = xin.tile([P, NDC, L], bf16)
            kt_s = xin.tile([P, NDC, L], bf16)
            vt_s = xin.tile([P, NDC, L], bf16)
            qt_r = qt_d.rearrange("(o p) l -> p o l", p=P)
            kt_r = kt_d.rearrange("(o p) l -> p o l", p=P)
            vt_r = vt_d.rearrange("(o p) l -> p o l", p=P)

            def load_chunk(sb, rr, c):
                nc.sync.dma_start(sb[:, :, ts(c, 512)], rr[:, :, ts(c, 512)])

            # critical-path-first load order
            nc.sync.dma_start(wq_s[:], wq_d.rearrange("(o p) e -> p o e", p=P))
            load_chunk(qt_s, qt_r, 0)
            nc.sync.dma_start(wk_s[:], wk_d.rearrange("(o p) e -> p o e", p=P))
            load_chunk(kt_s, kt_r, 0)
            load_chunk(qt_s, qt_r, 1)
            load_chunk(qt_s, qt_r, 2)
            load_chunk(qt_s, qt_r, 3)
            nc.sync.dma_start(wv_s[:], wv_d.rearrange("(o p) e -> p o e", p=P))
            load_chunk(vt_s, vt_r, 0)
            load_chunk(kt_s, kt_r, 1)
            load_chunk(vt_s, vt_r, 1)
            load_chunk(kt_s, kt_r, 2)
            load_chunk(vt_s, vt_r, 2)
            load_chunk(kt_s, kt_r, 3)
            load_chunk(vt_s, vt_r, 3)

            QT = proj.tile([P, L], bf16)
            KT = proj.tile([P, L], bf16)

            Dsum = stats.tile([P, 2, NKT, 2], f32)
            Dtot = stats.tile([P, 2, NKT], f32)
            Drec = stats.tile([P, 2, NKT], f32)

            # AV accumulator: out[q(128), qt(16), ev-pack(128)] f32 = 4 banks
            avp = avpsum.tile([P, NKT, P], f32)

            def q_proj_chunk(qc):
                ps = spsum.tile([P, 1024], f32, tag="sco", name="qproj")
                for dc in range(NDC):
                    nc.tensor.matmul(
                        ps[:, 0:512],
                        lhsT=wq_s[:, dc, :],
                        rhs=qt_s[:, dc, ts(qc, 512)],
                        start=(dc == 0),
                        stop=(dc == NDC - 1),
                    )
                nc.vector.tensor_copy(QT[:, ts(qc, 512)], ps[:, 0:512])

            def k_proj_chunk(c, ps):
                # kproj rides in cols 512:1024 of the given spsum tile
                for dc in range(NDC):
                    nc.tensor.matmul(
                        ps[:, 512:1024],
                        lhsT=wk_s[:, dc, :],
                        rhs=kt_s[:, dc, ts(c, 512)],
                        start=(dc == 0),
                        stop=(dc == NDC - 1),
                    )
                nc.scalar.copy(KT[:, ts(c, 512)], ps[:, 512:1024])

            # warmup: QT all 4 chunks (needed at kt=0), KT chunk 0
            q_proj_chunk(0)
            ps0 = spsum.tile([P, 1024], f32, tag="sco", name="kproj0")
            k_proj_chunk(0, ps0)
            q_proj_chunk(1)
            q_proj_chunk(2)
            q_proj_chunk(3)

            Etiles = {}

            def scores_exp(kt, h, half, E):
                hp = h * DH
                ps = spsum.tile([P, 1024], f32, tag="sco", name="sco")
                for j in range(2):
                    qc = half * 2 + j
                    nc.tensor.matmul(
                        ps[:, ts(j, 512)],
                        lhsT=KT[hp : hp + DH, ts(kt, P)],
                        rhs=QT[hp : hp + DH, ts(qc, 512)],
                        start=True,
                        stop=True,
                    )
                if (kt, h, half) in DVE_TILES:
                    e16 = E[:, ts(half, 1024)].bitcast(i16)
                    nc.vector.tensor_scalar(
                        e16, ps[:], EXP_A, EXP_B, ALU.mult, ALU.add
                    )
                    nc.vector.tensor_reduce(
                        Dsum[:, h : h + 1, kt : kt + 1, half : half + 1],
                        E[:, ts(half, 1024)],
                        axis=mybir.AxisListType.X,
                        op=ALU.add,
                    )
                else:
                    nc.scalar.activation(
                        E[:, ts(half, 1024)],
                        ps[:],
                        AF.Exp,
                        scale=SCALE,
                        accum_out=Dsum[:, h : h + 1, kt : kt + 1, half : half + 1],
                    )

            def av_block(kt):
                E0, E1 = Etiles[kt]
                Vs = Vstiles[kt]
                for qt in range(NKT):
                    for h in range(2):
                        E = E0 if h == 0 else E1
                        # start=True zeroes the whole PSUM bank (4 q-tiles),
                        # so only the first matmul per bank may carry it
                        nc.tensor.matmul(
                            avp[:, qt, ts(h, DH)],
                            lhsT=E[:, ts(qt, P)],
                            rhs=Vs[:, ts(h, DH)],
                            start=(kt == 0 and h == 0 and qt % 4 == 0),
                            stop=(kt == NKT - 1),
                            skip_group_check=True,
                        )

            Vstiles = {}

            for kt in range(NKT):
                E0 = epool.tile([P, L], bf16, tag="E", name=f"E{kt}h0")
                E1 = epool.tile([P, L], bf16, tag="E", name=f"E{kt}h1")
                Etiles[kt] = (E0, E1)
                scores_exp(kt, 0, 0, E0)
                scores_exp(kt, 0, 1, E0)
                scores_exp(kt, 1, 0, E1)
                scores_exp(kt, 1, 1, E1)

                # AV for the previous k-tile immediately after this kt's
                # scores: its deps (E/Vs of kt-1) are ready, so PE never
                # stalls on this kt's exps
                if kt > 0:
                    av_block(kt - 1)

                # V projection for this k-tile (+ a due K-proj chunk)
                psv = spsum.tile([P, 1024], f32, tag="sco", name="psv")
                for dc in range(NDC):
                    nc.tensor.matmul(
                        psv[:, 0:P],
                        lhsT=vt_s[:, dc, ts(kt, P)],
                        rhs=wv_s[:, dc, :],
                        start=(dc == 0),
                        stop=(dc == NDC - 1),
                    )
                if kt < 3:
                    k_proj_chunk(kt + 1, psv)

                # D and V~ = V/D
                nc.gpsimd.tensor_add(
                    Dtot[:, :, kt : kt + 1],
                    Dsum[:, :, kt : kt + 1, 0:1],
                    Dsum[:, :, kt : kt + 1, 1:2],
                )
                nc.vector.reciprocal(Drec[:, :, kt : kt + 1], Dtot[:, :, kt : kt + 1])
                Vs = vspool.tile([P, P], bf16, tag="vs")
                Vstiles[kt] = Vs
                for h in range(2):
                    nc.vector.tensor_scalar_mul(
                        Vs[:, ts(h, DH)],
                        psv[:, ts(h, DH)],
                        Drec[:, h : h + 1, kt : kt + 1],
                    )

            av_block(NKT - 1)

            # tail: evacuate AV psum + store, alternating DVE/ACT
            out_r = out_d.rearrange("(t p) e -> p t e", p=P)
            for c in range(4):
                oc = outp.tile([P, 4, P], f32, tag="oc")
                if c % 2 == 0:
                    nc.vector.tensor_copy(oc[:], avp[:, 4 * c : 4 * c + 4, :])
                else:
                    nc.scalar.copy(oc[:], avp[:, 4 * c : 4 * c + 4, :])
                nc.sync.dma_start(out_r[:, 4 * c : 4 * c + 4, :], oc[:])

    nc.compile()
    return nc


def _get_program():
    if "nc" not in _CACHE:
        _CACHE["nc"] = _build_program()
    return _CACHE["nc"]


def kernel(keys, queries, values, WQ, WK, WV):
    import ml_dtypes

    from concourse import bass_utils

    bf = ml_dtypes.bfloat16
    keys = np.asarray(keys)
    queries = np.asarray(queries)
    values = np.asarray(values)
    WQ = np.asarray(WQ)
    WK = np.asarray(WK)
    WV = np.asarray(WV)

    nc = _get_program()

    in_maps = []
    for c in range(N_CORES):
        b = c // 4
        h0 = 2 * (c % 4)
        h1 = h0 + 1
        in_maps.append(
            {
                "qt": np.ascontiguousarray(queries[b].T).astype(bf),
                "kt": np.ascontiguousarray(keys[b].T).astype(bf),
                "vt": np.ascontiguousarray(values[b].T).astype(bf),
                "wq": np.concatenate([WQ[h0], WQ[h1]], axis=1).astype(bf),
                "wk": np.concatenate([WK[h0], WK[h1]], axis=1).astype(bf),
                "wv": np.concatenate([WV[h0], WV[h1]], axis=1).astype(bf),
            }
        )

    res = bass_utils.run_bass_kernel_spmd(nc, in_maps, core_ids=list(range(N_CORES)))

    out = np.empty((B, L, H * DH), dtype=np.float32)
    for c in range(N_CORES):
        b = c // 4
        h0 = 2 * (c % 4)
        ot = res.results[c]["out"]  # [L, 128]
        out[b, :, h0 * DH : (h0 + 2) * DH] = ot
    return out


# revision 10
# speedup vs baseline: 1.0417x; 1.0417x over previous
"""MultiHeadAttention Trainium2 kernel (8 NeuronCores, SPMD).

Problem: B=2, L=2048, DK=DV=512, H=8, dh=64.
  Q = q @ WQ[h]; K = k @ WK[h]; V = v @ WV[h]       (per head)
  y = Q K^T / sqrt(L); z = softmax(y, axis=QUERY); out = z @ V
  concat heads on feature dim.

Sharding: 16 (b,h) pairs over 8 cores -> 2 heads (same batch) per core.

Device structure (per core, heads h0/h1):
  QT/KT = [e-pack(128), L] bf16; scores S[k-tile(128), q] in PSUM.
  exp is the bottleneck: split across ACT (exact Exp + fused accum row
  sums for D) and DVE (Schraudolph bf16-domain approx: one tensor_scalar
  computing round(score*A + B) -> int16, whose bit pattern read as bf16
  is exp(score*SCALE) to ~1.8% rel; D via tensor_reduce on the same
  tile).  1/D[k] is folded into V rows.
  AV uses E-tiles as the stationary operand: out[q(128), ev] accumulates
  over k-tiles in a single 4-bank PSUM region, 64 moving rows per
  matmul, so AV streams half the rows of the out^T[ev, q] layout.
Core output: [L, 128] f32 = two heads' outputs concatenated on the
feature dim; host slices it straight into the full output.
"""

import math

import numpy as np

B = 2
L = 2048
DK = 512
H = 8
DH = 64
P = 128
NKT = L // P  # 16 k-tiles
NDC = DK // P  # 4 d-chunks
N_CORES = 8

SCALE = 1.0 / math.sqrt(float(L))
# Schraudolph in bf16-bits domain: round(raw_score*EXP_A + EXP_B) as int16,
# bitcast bf16 ~= exp(raw_score*SCALE).  C calibrated against np.exp.
EXP_A = 128.0 * math.log2(math.e) * SCALE
EXP_B = 16256.0 - 12.0

# (kt, h) units whose exp runs on DVE (Schraudolph) instead of ACT.  Whole
# (kt, h) blocks only: approximating all q of a k-row keeps the Schraudolph
# bias common-mode in z = E/D so it cancels; mixing exact/approx halves in
# one row leaves a systematic bias (~2x the error).
DVE_HEADS = {(kt, 1) for kt in range(NKT)}

_CACHE = {}


def _build_program():
    import concourse.bass as bass
    import concourse.tile as tile
    from concourse import bacc, mybir
    from concourse.bass import ts

    f32 = mybir.dt.float32
    bf16 = mybir.dt.bfloat16
    i16 = mybir.dt.int16
    AF = mybir.ActivationFunctionType
    ALU = mybir.AluOpType

    nc = bacc.Bacc("TRN2", target_bir_lowering=False, debug=False)

    qt_d = nc.dram_tensor("qt", [DK, L], bf16, kind="ExternalInput")
    kt_d = nc.dram_tensor("kt", [DK, L], bf16, kind="ExternalInput")
    vt_d = nc.dram_tensor("vt", [DK, L], bf16, kind="ExternalInput")
    wq_d = nc.dram_tensor("wq", [DK, P], bf16, kind="ExternalInput")
    wk_d = nc.dram_tensor("wk", [DK, P], bf16, kind="ExternalInput")
    wv_d = nc.dram_tensor("wv", [DK, P], bf16, kind="ExternalInput")
    out_d = nc.dram_tensor("out", [L, P], f32, kind="ExternalOutput")

    with tile.TileContext(nc) as tc:
        with (
            tc.tile_pool(name="consts", bufs=1) as consts,
            tc.tile_pool(name="xin", bufs=1) as xin,
            tc.tile_pool(name="proj", bufs=1) as proj,
            tc.tile_pool(name="epool", bufs=4) as epool,
            tc.tile_pool(name="scrpool", bufs=2) as scrpool,
            tc.tile_pool(name="vspool", bufs=3) as vspool,
            tc.tile_pool(name="stats", bufs=1) as stats,
            tc.tile_pool(name="outp", bufs=2) as outp,
            tc.tile_pool(name="spsum", bufs=2, space="PSUM") as spsum,
            tc.tile_pool(name="avpsum", bufs=1, space="PSUM") as avpsum,
        ):
            wq_s = consts.tile([P, NDC, P], bf16)
            wk_s = consts.tile([P, NDC, P], bf16)
            wv_s = consts.tile([P, NDC, P], bf16)
            qt_s = xin.tile([P, NDC, L], bf16)
            kt_s = xin.tile([P, NDC, L], bf16)
            vt_s = xin.tile([P, NDC, L], bf16)
            qt_r = qt_d.rearrange("(o p) l -> p o l", p=P)
            kt_r = kt_d.rearrange("(o p) l -> p o l", p=P)
            vt_r = vt_d.rearrange("(o p) l -> p o l", p=P)

            def load_chunk(sb, rr, c):
                nc.sync.dma_start(sb[:, :, ts(c, 512)], rr[:, :, ts(c, 512)])

            # critical-path-first load order
            nc.sync.dma_start(wq_s[:], wq_d.rearrange("(o p) e -> p o e", p=P))
            load_chunk(qt_s, qt_r, 0)
            nc.sync.dma_start(wk_s[:], wk_d.rearrange("(o p) e -> p o e", p=P))
            load_chunk(kt_s, kt_r, 0)
            load_chunk(qt_s, qt_r, 1)
            load_chunk(qt_s, qt_r, 2)
            load_chunk(qt_s, qt_r, 3)
            nc.sync.dma_start(wv_s[:], wv_d.rearrange("(o p) e -> p o e", p=P))
            load_chunk(vt_s, vt_r, 0)
            load_chunk(kt_s, kt_r, 1)
            load_chunk(vt_s, vt_r, 1)
            load_chunk(kt_s, kt_r, 2)
            load_chunk(vt_s, vt_r, 2)
            load_chunk(kt_s, kt_r, 3)
            load_chunk(vt_s, vt_r, 3)

            QT = proj.tile([P, L], bf16)
            KT = proj.tile([P, L], bf16)

            Dsum = stats.tile([P, 2, NKT, 2], f32)
            # [P, kt*2 + h] layout: 2-D slices (required by tensor_scalar
            # accum_out) and per-kt head pairs stay adjacent for reciprocal
            Dtot = stats.tile([P, NKT * 2], f32)
            Drec = stats.tile([P, NKT * 2], f32)

            # AV accumulator: out[q(128), qt(16), ev-pack(128)] f32 = 4 banks
            avp = avpsum.tile([P, NKT, P], f32)

            def q_proj_chunk(qc):
                ps = spsum.tile([P, 1024], f32, tag="sco", name="qproj")
                for dc in range(NDC):
                    nc.tensor.matmul(
                        ps[:, 0:512],
                        lhsT=wq_s[:, dc, :],
                        rhs=qt_s[:, dc, ts(qc, 512)],
                        start=(dc == 0),
                        stop=(dc == NDC - 1),
                    )
                nc.vector.tensor_copy(QT[:, ts(qc, 512)], ps[:, 0:512])

            def k_proj_chunk(c, ps):
                # kproj rides in cols 512:1024 of the given spsum tile
                for dc in range(NDC):
                    nc.tensor.matmul(
                        ps[:, 512:1024],
                        lhsT=wk_s[:, dc, :],
                        rhs=kt_s[:, dc, ts(c, 512)],
                        start=(dc == 0),
                        stop=(dc == NDC - 1),
                    )
                nc.scalar.copy(KT[:, ts(c, 512)], ps[:, 512:1024])

            # warmup: QT all 4 chunks (needed at kt=0), KT chunk 0
            q_proj_chunk(0)
            ps0 = spsum.tile([P, 1024], f32, tag="sco", name="kproj0")
            k_proj_chunk(0, ps0)
            q_proj_chunk(1)
            q_proj_chunk(2)
            q_proj_chunk(3)

            Etiles = {}

            def scores_exp(kt, h, half, E):
                hp = h * DH
                ps = spsum.tile([P, 1024], f32, tag="sco", name="sco")
                for j in range(2):
                    qc = half * 2 + j
                    nc.tensor.matmul(
                        ps[:, ts(j, 512)],
                        lhsT=KT[hp : hp + DH, ts(kt, P)],
                        rhs=QT[hp : hp + DH, ts(qc, 512)],
                        start=True,
                        stop=True,
                    )
                if (kt, h) in DVE_HEADS:
                    e16 = E[:, ts(half, 1024)].bitcast(i16)
                    nc.vector.tensor_scalar(
                        e16, ps[:], EXP_A, EXP_B, ALU.mult, ALU.add
                    )
                else:
                    nc.scalar.activation(
                        E[:, ts(half, 1024)],
                        ps[:],
                        AF.Exp,
                        scale=SCALE,
                        accum_out=Dsum[:, h : h + 1, kt : kt + 1, half : half + 1],
                    )

            def av_block(kt):
                E0, E1 = Etiles[kt]
                Vs = Vstiles[kt]
                for qt in range(NKT):
                    for h in range(2):
                        E = E0 if h == 0 else E1
                        # start=True zeroes the whole PSUM bank (4 q-tiles),
                        # so only the first matmul per bank may carry it
                        nc.tensor.matmul(
                            avp[:, qt, ts(h, DH)],
                            lhsT=E[:, ts(qt, P)],
                            rhs=Vs[:, ts(h, DH)],
                            start=(kt == 0 and h == 0 and qt % 4 == 0),
                            stop=(kt == NKT - 1),
                            skip_group_check=True,
                        )

            Vstiles = {}

            for kt in range(NKT):
                E0 = epool.tile([P, L], bf16, tag="E", name=f"E{kt}h0")
                E1 = epool.tile([P, L], bf16, tag="E", name=f"E{kt}h1")
                Etiles[kt] = (E0, E1)
                scores_exp(kt, 0, 0, E0)
                scores_exp(kt, 0, 1, E0)
                scores_exp(kt, 1, 0, E1)
                scores_exp(kt, 1, 1, E1)

                # AV for the previous k-tile immediately after this kt's
                # scores: its deps (E/Vs of kt-1) are ready, so PE never
                # stalls on this kt's exps
                if kt > 0:
                    av_block(kt - 1)

                # V projection for this k-tile (+ a due K-proj chunk)
                psv = spsum.tile([P, 1024], f32, tag="sco", name="psv")
                for dc in range(NDC):
                    nc.tensor.matmul(
                        psv[:, 0:P],
                        lhsT=vt_s[:, dc, ts(kt, P)],
                        rhs=wv_s[:, dc, :],
                        start=(dc == 0),
                        stop=(dc == NDC - 1),
                    )
                if kt < 3:
                    k_proj_chunk(kt + 1, psv)

                # D and V~ = V/D
                for h in range(2):
                    if (kt, h) in DVE_HEADS:
                        # one 4x-mode bf16 copy with fused row-sum over the
                        # whole 2048-q row gives Dtot for this head directly
                        E = Etiles[kt][h]
                        scr = scrpool.tile([P, L], bf16, tag="scr")
                        nc.vector.tensor_scalar(
                            scr[:],
                            E[:],
                            1.0,
                            0.0,
                            ALU.mult,
                            ALU.add,
                            accum_out=Dtot[:, 2 * kt + h : 2 * kt + h + 1],
                        )
                    else:
                        nc.vector.tensor_add(
                            Dtot[:, 2 * kt + h : 2 * kt + h + 1],
                            Dsum[:, h, kt : kt + 1, 0],
                            Dsum[:, h, kt : kt + 1, 1],
                        )
                nc.vector.reciprocal(
                    Drec[:, 2 * kt : 2 * kt + 2], Dtot[:, 2 * kt : 2 * kt + 2]
                )
                Vs = vspool.tile([P, P], bf16, tag="vs")
                Vstiles[kt] = Vs
                for h in range(2):
                    # per-partition 1/D scale on ACT (fills ACT gaps)
                    nc.scalar.mul(
                        Vs[:, ts(h, DH)],
                        psv[:, ts(h, DH)],
                        Drec[:, 2 * kt + h : 2 * kt + h + 1],
                    )

            av_block(NKT - 1)

            # tail: evacuate AV psum + store, alternating DVE/ACT
            out_r = out_d.rearrange("(t p) e -> p t e", p=P)
            for c in range(4):
                oc = outp.tile([P, 4, P], f32, tag="oc")
                if c % 2 == 0:
                    nc.vector.tensor_copy(oc[:], avp[:, 4 * c : 4 * c + 4, :])
                else:
                    nc.scalar.copy(oc[:], avp[:, 4 * c : 4 * c + 4, :])
                nc.sync.dma_start(out_r[:, 4 * c : 4 * c + 4, :], oc[:])

    nc.compile()
    return nc


def _get_program():
    if "nc" not in _CACHE:
        _CACHE["nc"] = _build_program()
    return _CACHE["nc"]


def kernel(keys, queries, values, WQ, WK, WV):
    import ml_dtypes

    from concourse import bass_utils

    bf = ml_dtypes.bfloat16
    keys = np.asarray(keys)
    queries = np.asarray(queries)
    values = np.asarray(values)
    WQ = np.asarray(WQ)
    WK = np.asarray(WK)
    WV = np.asarray(WV)

    nc = _get_program()

    in_maps = []
    for c in range(N_CORES):
        b = c // 4
        h0 = 2 * (c % 4)
        h1 = h0 + 1
        in_maps.append(
            {
                "qt": np.ascontiguousarray(queries[b].T).astype(bf),
                "kt": np.ascontiguousarray(keys[b].T).astype(bf),
                "vt": np.ascontiguousarray(values[b].T).astype(bf),
                "wq": np.concatenate([WQ[h0], WQ[h1]], axis=1).astype(bf),
                "wk": np.concatenate([WK[h0], WK[h1]], axis=1).astype(bf),
                "wv": np.concatenate([WV[h0], WV[h1]], axis=1).astype(bf),
            }
        )

    res = bass_utils.run_bass_kernel_spmd(nc, in_maps, core_ids=list(range(N_CORES)))

    out = np.empty((B, L, H * DH), dtype=np.float32)
    for c in range(N_CORES):
        b = c // 4
        h0 = 2 * (c % 4)
        ot = res.results[c]["out"]  # [L, 128]
        out[b, :, h0 * DH : (h0 + 2) * DH] = ot
    return out


# revision 11
# speedup vs baseline: 1.2014x; 1.1533x over previous
"""MultiHeadAttention Trainium2 kernel (8 NeuronCores, SPMD).

Problem: B=2, L=2048, DK=DV=512, H=8, dh=64.
  Q = q @ WQ[h]; K = k @ WK[h]; V = v @ WV[h]       (per head)
  y = Q K^T / sqrt(L); z = softmax(y, axis=QUERY); out = z @ V
  concat heads on feature dim.

Sharding: 16 (b,h) pairs over 8 cores -> 2 heads (same batch) per core.

Device structure (per core, heads h0/h1):
  QT/KT = [e-pack(128), L] bf16; scores S[k-tile(128), q] in PSUM.
  exp is the bottleneck: split across ACT (exact Exp + fused accum row
  sums for D) and DVE (Schraudolph bf16-domain approx: one tensor_scalar
  computing round(score*A + B) -> int16, whose bit pattern read as bf16
  is exp(score*SCALE) to ~1.8% rel; D via tensor_reduce on the same
  tile).  1/D[k] is folded into V rows.
  AV uses E-tiles as the stationary operand: out[q(128), ev] accumulates
  over k-tiles in a single 4-bank PSUM region, 64 moving rows per
  matmul, so AV streams half the rows of the out^T[ev, q] layout.
Core output: [L, 128] f32 = two heads' outputs concatenated on the
feature dim; host slices it straight into the full output.
"""

import math

import numpy as np

B = 2
L = 2048
DK = 512
H = 8
DH = 64
P = 128
NKT = L // P  # 16 k-tiles
NDC = DK // P  # 4 d-chunks
N_CORES = 8

SCALE = 1.0 / math.sqrt(float(L))
# Schraudolph in bf16-bits domain: round(raw_score*EXP_A + EXP_B) as int16,
# bitcast bf16 ~= exp(raw_score*SCALE).  C calibrated against np.exp.
EXP_A = 128.0 * math.log2(math.e) * SCALE
EXP_B = 16256.0 - 12.0

# (kt, h) units whose exp runs on DVE (Schraudolph) instead of ACT.  Whole
# (kt, h) blocks only: approximating all q of a k-row keeps the Schraudolph
# bias common-mode in z = E/D so it cancels; mixing exact/approx halves in
# one row leaves a systematic bias (~2x the error).
DVE_HEADS = {(kt, 1) for kt in range(NKT)}

_CACHE = {}


def _build_program():
    import concourse.bass as bass
    import concourse.tile as tile
    from concourse import bacc, mybir
    from concourse.bass import ts

    f32 = mybir.dt.float32
    bf16 = mybir.dt.bfloat16
    i16 = mybir.dt.int16
    AF = mybir.ActivationFunctionType
    ALU = mybir.AluOpType

    nc = bacc.Bacc("TRN2", target_bir_lowering=False, debug=False)

    qt_d = nc.dram_tensor("qt", [DK, L], bf16, kind="ExternalInput")
    kt_d = nc.dram_tensor("kt", [DK, L], bf16, kind="ExternalInput")
    vt_d = nc.dram_tensor("vt", [DK, L], bf16, kind="ExternalInput")
    wq_d = nc.dram_tensor("wq", [DK, P], bf16, kind="ExternalInput")
    wk_d = nc.dram_tensor("wk", [DK, P], bf16, kind="ExternalInput")
    wv_d = nc.dram_tensor("wv", [DK, P], bf16, kind="ExternalInput")
    out_d = nc.dram_tensor("out", [L, P], f32, kind="ExternalOutput")

    with tile.TileContext(nc) as tc:
        with (
            tc.tile_pool(name="consts", bufs=1) as consts,
            tc.tile_pool(name="xin", bufs=1) as xin,
            tc.tile_pool(name="proj", bufs=1) as proj,
            tc.tile_pool(name="epool", bufs=6) as epool,
            tc.tile_pool(name="scrpool", bufs=2) as scrpool,
            tc.tile_pool(name="vfpool", bufs=4) as vfpool,
            tc.tile_pool(name="vspool", bufs=4) as vspool,
            tc.tile_pool(name="stats", bufs=1) as stats,
            tc.tile_pool(name="outp", bufs=2) as outp,
            tc.tile_pool(name="spsum", bufs=2, space="PSUM") as spsum,
            tc.tile_pool(name="avpsum", bufs=1, space="PSUM") as avpsum,
        ):
            wq_s = consts.tile([P, NDC, P], bf16)
            wk_s = consts.tile([P, NDC, P], bf16)
            wv_s = consts.tile([P, NDC, P], bf16)
            qt_s = xin.tile([P, NDC, L], bf16)
            kt_s = xin.tile([P, NDC, L], bf16)
            vt_s = xin.tile([P, NDC, L], bf16)
            qt_r = qt_d.rearrange("(o p) l -> p o l", p=P)
            kt_r = kt_d.rearrange("(o p) l -> p o l", p=P)
            vt_r = vt_d.rearrange("(o p) l -> p o l", p=P)

            def load_chunk(sb, rr, c):
                nc.sync.dma_start(sb[:, :, ts(c, 512)], rr[:, :, ts(c, 512)])

            # critical-path-first load order
            nc.sync.dma_start(wq_s[:], wq_d.rearrange("(o p) e -> p o e", p=P))
            load_chunk(qt_s, qt_r, 0)
            nc.sync.dma_start(wk_s[:], wk_d.rearrange("(o p) e -> p o e", p=P))
            load_chunk(kt_s, kt_r, 0)
            load_chunk(qt_s, qt_r, 1)
            load_chunk(qt_s, qt_r, 2)
            load_chunk(qt_s, qt_r, 3)
            nc.sync.dma_start(wv_s[:], wv_d.rearrange("(o p) e -> p o e", p=P))
            load_chunk(vt_s, vt_r, 0)
            load_chunk(kt_s, kt_r, 1)
            load_chunk(vt_s, vt_r, 1)
            load_chunk(kt_s, kt_r, 2)
            load_chunk(vt_s, vt_r, 2)
            load_chunk(kt_s, kt_r, 3)
            load_chunk(vt_s, vt_r, 3)

            QT = proj.tile([P, L], bf16)
            KT = proj.tile([P, L], bf16)

            Dsum = stats.tile([P, 2, NKT, 2], f32)
            # [P, kt*2 + h] layout: 2-D slices (required by tensor_scalar
            # accum_out) and per-kt head pairs stay adjacent for reciprocal
            Dtot = stats.tile([P, NKT * 2], f32)
            Drec = stats.tile([P, NKT * 2], f32)

            # AV accumulator: out[q(128), qt(16), ev-pack(128)] f32 = 4 banks
            avp = avpsum.tile([P, NKT, P], f32)

            def q_proj_chunk(qc):
                ps = spsum.tile([P, 1024], f32, tag="sco", name="qproj")
                for dc in range(NDC):
                    nc.tensor.matmul(
                        ps[:, 0:512],
                        lhsT=wq_s[:, dc, :],
                        rhs=qt_s[:, dc, ts(qc, 512)],
                        start=(dc == 0),
                        stop=(dc == NDC - 1),
                    )
                nc.vector.tensor_copy(QT[:, ts(qc, 512)], ps[:, 0:512])

            def k_proj_chunk(c, ps):
                # kproj rides in cols 512:1024 of the given spsum tile
                for dc in range(NDC):
                    nc.tensor.matmul(
                        ps[:, 512:1024],
                        lhsT=wk_s[:, dc, :],
                        rhs=kt_s[:, dc, ts(c, 512)],
                        start=(dc == 0),
                        stop=(dc == NDC - 1),
                    )
                nc.scalar.copy(KT[:, ts(c, 512)], ps[:, 512:1024])

            # warmup: QT all 4 chunks (needed at kt=0), KT chunk 0
            q_proj_chunk(0)
            ps0 = spsum.tile([P, 1024], f32, tag="sco", name="kproj0")
            k_proj_chunk(0, ps0)
            q_proj_chunk(1)
            q_proj_chunk(2)
            q_proj_chunk(3)

            Etiles = {}

            def scores_exp(kt, h, half, E):
                hp = h * DH
                ps = spsum.tile([P, 1024], f32, tag="sco", name="sco")
                for j in range(2):
                    qc = half * 2 + j
                    nc.tensor.matmul(
                        ps[:, ts(j, 512)],
                        lhsT=KT[hp : hp + DH, ts(kt, P)],
                        rhs=QT[hp : hp + DH, ts(qc, 512)],
                        start=True,
                        stop=True,
                    )
                if (kt, h) in DVE_HEADS:
                    e16 = E[:, ts(half, 1024)].bitcast(i16)
                    nc.vector.tensor_scalar(
                        e16, ps[:], EXP_A, EXP_B, ALU.mult, ALU.add
                    )
                else:
                    nc.scalar.activation(
                        E[:, ts(half, 1024)],
                        ps[:],
                        AF.Exp,
                        scale=SCALE,
                        accum_out=Dsum[:, h : h + 1, kt : kt + 1, half : half + 1],
                    )

            def av_block(kt):
                E0, E1 = Etiles[kt]
                Vs = Vstiles[kt]
                for qt in range(NKT):
                    for h in range(2):
                        E = E0 if h == 0 else E1
                        # start=True zeroes the whole PSUM bank (4 q-tiles),
                        # so only the first matmul per bank may carry it
                        nc.tensor.matmul(
                            avp[:, qt, ts(h, DH)],
                            lhsT=E[:, ts(qt, P)],
                            rhs=Vs[:, ts(h, DH)],
                            start=(kt == 0 and h == 0 and qt % 4 == 0),
                            stop=(kt == NKT - 1),
                            skip_group_check=True,
                        )

            Vstiles = {}

            for kt in range(NKT):
                E0 = epool.tile([P, L], bf16, tag="E", name=f"E{kt}h0")
                E1 = epool.tile([P, L], bf16, tag="E", name=f"E{kt}h1")
                Etiles[kt] = (E0, E1)
                scores_exp(kt, 0, 0, E0)
                scores_exp(kt, 0, 1, E0)
                scores_exp(kt, 1, 0, E1)
                scores_exp(kt, 1, 1, E1)

                # AV for the previous k-tile immediately after this kt's
                # scores: its deps (E/Vs of kt-1) are ready, so PE never
                # stalls on this kt's exps
                if kt > 0:
                    av_block(kt - 1)

                # V projection for this k-tile (+ a due K-proj chunk)
                psv = spsum.tile([P, 1024], f32, tag="sco", name="psv")
                for dc in range(NDC):
                    nc.tensor.matmul(
                        psv[:, 0:P],
                        lhsT=vt_s[:, dc, ts(kt, P)],
                        rhs=wv_s[:, dc, :],
                        start=(dc == 0),
                        stop=(dc == NDC - 1),
                    )
                if kt < 3:
                    k_proj_chunk(kt + 1, psv)
                # evacuate raw V immediately (no D dependency) so the psum
                # rotation slot frees fast; scale from SBUF later
                Vf = vfpool.tile([P, P], bf16, tag="vf")
                nc.scalar.copy(Vf[:], psv[:, 0:P])

                # D and V~ = V/D
                for h in range(2):
                    if (kt, h) in DVE_HEADS:
                        # one 4x-mode bf16 copy with fused row-sum over the
                        # whole 2048-q row gives Dtot for this head directly
                        E = Etiles[kt][h]
                        scr = scrpool.tile([P, L], bf16, tag="scr")
                        nc.vector.tensor_scalar(
                            scr[:],
                            E[:],
                            1.0,
                            0.0,
                            ALU.mult,
                            ALU.add,
                            accum_out=Dtot[:, 2 * kt + h : 2 * kt + h + 1],
                        )
                    else:
                        nc.gpsimd.tensor_add(
                            Dtot[:, 2 * kt + h : 2 * kt + h + 1],
                            Dsum[:, h, kt : kt + 1, 0],
                            Dsum[:, h, kt : kt + 1, 1],
                        )
                nc.vector.reciprocal(
                    Drec[:, 2 * kt : 2 * kt + 2], Dtot[:, 2 * kt : 2 * kt + 2]
                )
                Vs = vspool.tile([P, P], bf16, tag="vs")
                Vstiles[kt] = Vs
                for h in range(2):
                    # 4x-mode bf16 SBUF scale: V~ = V * (1/D) per partition
                    nc.vector.tensor_scalar_mul(
                        Vs[:, ts(h, DH)],
                        Vf[:, ts(h, DH)],
                        Drec[:, 2 * kt + h : 2 * kt + h + 1],
                    )

            av_block(NKT - 1)

            # tail: evacuate AV psum + store, alternating DVE/ACT
            out_r = out_d.rearrange("(t p) e -> p t e", p=P)
            for c in range(4):
                oc = outp.tile([P, 4, P], f32, tag="oc")
                if c % 2 == 0:
                    nc.vector.tensor_copy(oc[:], avp[:, 4 * c : 4 * c + 4, :])
                else:
                    nc.scalar.copy(oc[:], avp[:, 4 * c : 4 * c + 4, :])
                nc.sync.dma_start(out_r[:, 4 * c : 4 * c + 4, :], oc[:])

    nc.compile()
    return nc


def _get_program():
    if "nc" not in _CACHE:
        _CACHE["nc"] = _build_program()
    return _CACHE["nc"]


def kernel(keys, queries, values, WQ, WK, WV):
    import ml_dtypes

    from concourse import bass_utils

    bf = ml_dtypes.bfloat16
    keys = np.asarray(keys)
    queries = np.asarray(queries)
    values = np.asarray(values)
    WQ = np.asarray(WQ)
    WK = np.asarray(WK)
    WV = np.asarray(WV)

    nc = _get_program()

    in_maps = []
    for c in range(N_CORES):
        b = c // 4
        h0 = 2 * (c % 4)
        h1 = h0 + 1
        in_maps.append(
            {
                "qt": np.ascontiguousarray(queries[b].T).astype(bf),
                "kt": np.ascontiguousarray(keys[b].T).astype(bf),
                "vt": np.ascontiguousarray(values[b].T).astype(bf),
                "wq": np.concatenate([WQ[h0], WQ[h1]], axis=1).astype(bf),
                "wk": np.concatenate([WK[h0], WK[h1]], axis=1).astype(bf),
                "wv": np.concatenate([WV[h0], WV[h1]], axis=1).astype(bf),
            }
        )

    res = bass_utils.run_bass_kernel_spmd(nc, in_maps, core_ids=list(range(N_CORES)))

    out = np.empty((B, L, H * DH), dtype=np.float32)
    for c in range(N_CORES):
        b = c // 4
        h0 = 2 * (c % 4)
        ot = res.results[c]["out"]  # [L, 128]
        out[b, :, h0 * DH : (h0 + 2) * DH] = ot
    return out


# revision 12
# speedup vs baseline: 1.3338x; 1.1102x over previous
"""MultiHeadAttention Trainium2 kernel (8 NeuronCores, SPMD).

Problem: B=2, L=2048, DK=DV=512, H=8, dh=64.
  Q = q @ WQ[h]; K = k @ WK[h]; V = v @ WV[h]       (per head)
  y = Q K^T / sqrt(L); z = softmax(y, axis=QUERY); out = z @ V
  concat heads on feature dim.

Sharding: 16 (b,h) pairs over 8 cores -> 2 heads (same batch) per core.

Device structure (per core, heads h0/h1):
  QT/KT = [e-pack(128), L] bf16; scores S[k-tile(128), q] in PSUM as
  [128, 512] quarter-tiles (4 PSUM bufs -> deep cross-engine pipeline).
  exp is the bottleneck and is split: ACT takes h0 (exact Exp), DVE
  takes h1 via a Schraudolph bf16-domain approx (tensor_scalar
  round(score*A + B) -> int16; the bit pattern read as bf16 is
  exp(score*SCALE) to ~2% rel).  Entire (kt,h) blocks only, so the
  approx bias is common-mode in z = E/D and cancels.
  D row sums: one 4x-mode bf16 copy-with-accum over each head's
  [128, 2048] E row block (DVE).  1/D[k] is folded into V rows.
  AV uses E-tiles as the stationary operand: out[q(128), ev] accumulates
  over k-tiles in a single 4-bank PSUM region, 64 moving rows per
  matmul (half the streamed rows of the out^T[ev, q] layout).
Core output: [L, 128] f32 = two heads' outputs concatenated on the
feature dim; host slices it straight into the full output.
"""

import math

import numpy as np

B = 2
L = 2048
DK = 512
H = 8
DH = 64
P = 128
NKT = L // P  # 16 k-tiles
NDC = DK // P  # 4 d-chunks
N_CORES = 8

SCALE = 1.0 / math.sqrt(float(L))
# Schraudolph in bf16-bits domain: round(raw_score*EXP_A + EXP_B) as int16,
# bitcast bf16 ~= exp(raw_score*SCALE).  C calibrated against np.exp.
EXP_A = 128.0 * math.log2(math.e) * SCALE
EXP_B = 16256.0 - 12.0

# kt values whose h1 exp runs on DVE (Schraudolph) instead of ACT.  Whole
# (kt, h) blocks only: approximating all q of a k-row keeps the Schraudolph
# bias common-mode in z = E/D so it cancels.
DVE_KTS = set(range(NKT))

_CACHE = {}


def _build_program():
    import concourse.bass as bass
    import concourse.tile as tile
    from concourse import bacc, mybir
    from concourse.bass import ts

    f32 = mybir.dt.float32
    bf16 = mybir.dt.bfloat16
    i16 = mybir.dt.int16
    AF = mybir.ActivationFunctionType
    ALU = mybir.AluOpType

    nc = bacc.Bacc("TRN2", target_bir_lowering=False, debug=False)

    qt_d = nc.dram_tensor("qt", [DK, L], bf16, kind="ExternalInput")
    kt_d = nc.dram_tensor("kt", [DK, L], bf16, kind="ExternalInput")
    vt_d = nc.dram_tensor("vt", [DK, L], bf16, kind="ExternalInput")
    wq_d = nc.dram_tensor("wq", [DK, P], bf16, kind="ExternalInput")
    wk_d = nc.dram_tensor("wk", [DK, P], bf16, kind="ExternalInput")
    wv_d = nc.dram_tensor("wv", [DK, P], bf16, kind="ExternalInput")
    out_d = nc.dram_tensor("out", [L, P], f32, kind="ExternalOutput")

    with tile.TileContext(nc) as tc:
        with (
            tc.tile_pool(name="consts", bufs=1) as consts,
            tc.tile_pool(name="xin", bufs=1) as xin,
            tc.tile_pool(name="proj", bufs=1) as proj,
            tc.tile_pool(name="epool", bufs=6) as epool,
            tc.tile_pool(name="scrpool", bufs=2) as scrpool,
            tc.tile_pool(name="vfpool", bufs=4) as vfpool,
            tc.tile_pool(name="vspool", bufs=4) as vspool,
            tc.tile_pool(name="stats", bufs=1) as stats,
            tc.tile_pool(name="outp", bufs=2) as outp,
            tc.tile_pool(name="spsum", bufs=4, space="PSUM") as spsum,
            tc.tile_pool(name="avpsum", bufs=1, space="PSUM") as avpsum,
        ):
            wq_s = consts.tile([P, NDC, P], bf16)
            wk_s = consts.tile([P, NDC, P], bf16)
            wv_s = consts.tile([P, NDC, P], bf16)
            qt_s = xin.tile([P, NDC, L], bf16)
            kt_s = xin.tile([P, NDC, L], bf16)
            vt_s = xin.tile([P, NDC, L], bf16)
            qt_r = qt_d.rearrange("(o p) l -> p o l", p=P)
            kt_r = kt_d.rearrange("(o p) l -> p o l", p=P)
            vt_r = vt_d.rearrange("(o p) l -> p o l", p=P)

            def load_chunk(sb, rr, c):
                nc.sync.dma_start(sb[:, :, ts(c, 512)], rr[:, :, ts(c, 512)])

            # critical-path-first load order
            nc.sync.dma_start(wq_s[:], wq_d.rearrange("(o p) e -> p o e", p=P))
            load_chunk(qt_s, qt_r, 0)
            nc.sync.dma_start(wk_s[:], wk_d.rearrange("(o p) e -> p o e", p=P))
            load_chunk(kt_s, kt_r, 0)
            load_chunk(qt_s, qt_r, 1)
            load_chunk(qt_s, qt_r, 2)
            load_chunk(qt_s, qt_r, 3)
            nc.sync.dma_start(wv_s[:], wv_d.rearrange("(o p) e -> p o e", p=P))
            load_chunk(vt_s, vt_r, 0)
            load_chunk(kt_s, kt_r, 1)
            load_chunk(vt_s, vt_r, 1)
            load_chunk(kt_s, kt_r, 2)
            load_chunk(vt_s, vt_r, 2)
            load_chunk(kt_s, kt_r, 3)
            load_chunk(vt_s, vt_r, 3)

            QT = proj.tile([P, L], bf16)
            KT = proj.tile([P, L], bf16)

            # [P, kt*2 + h] layout: 2-D slices (required by tensor_scalar
            # accum_out) and per-kt head pairs stay adjacent for reciprocal
            Dtot = stats.tile([P, NKT * 2], f32)
            Drec = stats.tile([P, NKT * 2], f32)

            # AV accumulator: out[q(128), qt(16), ev-pack(128)] f32 = 4 banks
            avp = avpsum.tile([P, NKT, P], f32)

            def q_proj_chunk(qc):
                ps = spsum.tile([P, 512], f32, tag="sco", name="qproj")
                for dc in range(NDC):
                    nc.tensor.matmul(
                        ps[:],
                        lhsT=wq_s[:, dc, :],
                        rhs=qt_s[:, dc, ts(qc, 512)],
                        start=(dc == 0),
                        stop=(dc == NDC - 1),
                    )
                nc.vector.tensor_copy(QT[:, ts(qc, 512)], ps[:])

            def k_proj_chunk(c):
                ps = spsum.tile([P, 512], f32, tag="sco", name="kproj")
                for dc in range(NDC):
                    nc.tensor.matmul(
                        ps[:],
                        lhsT=wk_s[:, dc, :],
                        rhs=kt_s[:, dc, ts(c, 512)],
                        start=(dc == 0),
                        stop=(dc == NDC - 1),
                    )
                nc.scalar.copy(KT[:, ts(c, 512)], ps[:])

            # warmup: QT all 4 chunks (needed at kt=0), KT chunk 0
            q_proj_chunk(0)
            k_proj_chunk(0)
            q_proj_chunk(1)
            q_proj_chunk(2)
            q_proj_chunk(3)

            Etiles = {}
            Vstiles = {}

            def scores_exp(kt, h, qc, E):
                # one [128, 512] quarter: scores matmul + exp
                hp = h * DH
                ps = spsum.tile([P, 512], f32, tag="sco", name="sco")
                nc.tensor.matmul(
                    ps[:],
                    lhsT=KT[hp : hp + DH, ts(kt, P)],
                    rhs=QT[hp : hp + DH, ts(qc, 512)],
                    start=True,
                    stop=True,
                )
                if h == 1 and kt in DVE_KTS:
                    e16 = E[:, ts(qc, 512)].bitcast(i16)
                    nc.vector.tensor_scalar(
                        e16, ps[:], EXP_A, EXP_B, ALU.mult, ALU.add
                    )
                else:
                    nc.scalar.activation(E[:, ts(qc, 512)], ps[:], AF.Exp, scale=SCALE)

            def av_block(kt):
                E0, E1 = Etiles[kt]
                Vs = Vstiles[kt]
                for qt in range(NKT):
                    for h in range(2):
                        E = E0 if h == 0 else E1
                        # start=True zeroes the whole PSUM bank (4 q-tiles),
                        # so only the first matmul per bank may carry it
                        nc.tensor.matmul(
                            avp[:, qt, ts(h, DH)],
                            lhsT=E[:, ts(qt, P)],
                            rhs=Vs[:, ts(h, DH)],
                            start=(kt == 0 and h == 0 and qt % 4 == 0),
                            stop=(kt == NKT - 1),
                            skip_group_check=True,
                        )

            for kt in range(NKT):
                E0 = epool.tile([P, L], bf16, tag="E", name=f"E{kt}h0")
                E1 = epool.tile([P, L], bf16, tag="E", name=f"E{kt}h1")
                Etiles[kt] = (E0, E1)
                for qc in range(4):
                    scores_exp(kt, 0, qc, E0)
                for qc in range(4):
                    scores_exp(kt, 1, qc, E1)

                # AV for the previous k-tile (deps all ready: PE never
                # stalls on this kt's exps)
                if kt > 0:
                    av_block(kt - 1)

                # V projection for this k-tile; evacuate raw V immediately
                # (no D dependency) so the psum slot frees fast
                psv = spsum.tile([P, 512], f32, tag="sco", name="psv")
                for dc in range(NDC):
                    nc.tensor.matmul(
                        psv[:, 0:P],
                        lhsT=vt_s[:, dc, ts(kt, P)],
                        rhs=wv_s[:, dc, :],
                        start=(dc == 0),
                        stop=(dc == NDC - 1),
                    )
                Vf = vfpool.tile([P, P], bf16, tag="vf")
                nc.scalar.copy(Vf[:], psv[:, 0:P])
                if kt < 3:
                    k_proj_chunk(kt + 1)

                # D per head: one 4x-mode bf16 copy with fused row-sum over
                # the whole 2048-q row block
                for h in range(2):
                    E = Etiles[kt][h]
                    scr = scrpool.tile([P, L], bf16, tag="scr")
                    nc.vector.tensor_scalar(
                        scr[:],
                        E[:],
                        1.0,
                        0.0,
                        ALU.mult,
                        ALU.add,
                        accum_out=Dtot[:, 2 * kt + h : 2 * kt + h + 1],
                    )
                nc.vector.reciprocal(
                    Drec[:, 2 * kt : 2 * kt + 2], Dtot[:, 2 * kt : 2 * kt + 2]
                )
                Vs = vspool.tile([P, P], bf16, tag="vs")
                Vstiles[kt] = Vs
                for h in range(2):
                    # per-partition 1/D scale on ACT
                    nc.scalar.mul(
                        Vs[:, ts(h, DH)],
                        Vf[:, ts(h, DH)],
                        Drec[:, 2 * kt + h : 2 * kt + h + 1],
                    )

            av_block(NKT - 1)

            # tail: evacuate AV psum + store, alternating DVE/ACT
            out_r = out_d.rearrange("(t p) e -> p t e", p=P)
            for c in range(4):
                oc = outp.tile([P, 4, P], f32, tag="oc")
                if c % 2 == 0:
                    nc.vector.tensor_copy(oc[:], avp[:, 4 * c : 4 * c + 4, :])
                else:
                    nc.scalar.copy(oc[:], avp[:, 4 * c : 4 * c + 4, :])
                nc.sync.dma_start(out_r[:, 4 * c : 4 * c + 4, :], oc[:])

    nc.compile()
    return nc


def _get_program():
    if "nc" not in _CACHE:
        _CACHE["nc"] = _build_program()
    return _CACHE["nc"]


def kernel(keys, queries, values, WQ, WK, WV):
    import ml_dtypes

    from concourse import bass_utils

    bf = ml_dtypes.bfloat16
    keys = np.asarray(keys)
    queries = np.asarray(queries)
    values = np.asarray(values)
    WQ = np.asarray(WQ)
    WK = np.asarray(WK)
    WV = np.asarray(WV)

    nc = _get_program()

    in_maps = []
    for c in range(N_CORES):
        b = c // 4
        h0 = 2 * (c % 4)
        h1 = h0 + 1
        in_maps.append(
            {
                "qt": np.ascontiguousarray(queries[b].T).astype(bf),
                "kt": np.ascontiguousarray(keys[b].T).astype(bf),
                "vt": np.ascontiguousarray(values[b].T).astype(bf),
                "wq": np.concatenate([WQ[h0], WQ[h1]], axis=1).astype(bf),
                "wk": np.concatenate([WK[h0], WK[h1]], axis=1).astype(bf),
                "wv": np.concatenate([WV[h0], WV[h1]], axis=1).astype(bf),
            }
        )

    res = bass_utils.run_bass_kernel_spmd(nc, in_maps, core_ids=list(range(N_CORES)))

    out = np.empty((B, L, H * DH), dtype=np.float32)
    for c in range(N_CORES):
        b = c // 4
        h0 = 2 * (c % 4)
        ot = res.results[c]["out"]  # [L, 128]
        out[b, :, h0 * DH : (h0 + 2) * DH] = ot
    return out
